# revision 23
# baseline (speedup 1.0000x reference)
"""LlamaSkipMLP Trainium2 kernel.

Strategy: data-parallel over the token dim across 8 NeuronCores (no
collectives).  Each core computes out_c = silu(x_c@Wg'.T) * (x_c@Wu'.T) @ Wd'.T
for its 1024-token slice, where Wg'/Wu'/Wd' are the active-neuron
gather of the weights (done host-side; for active_idx = arange(k) it
is a plain slice).

Device kernel (per core, Tile framework):
  phase 1: g/u GEMMs contract hidden dim H (on PE partitions), fused
           SiLU*up on ACT+DVE, h stored [k_part, t_free] in fp16.
  phase 2: down GEMM contracts the active-neuron dim k; h tiles serve
           as the stationary operand, W_down^T tiles as the moving
           operand, so the output lands as [t_part, h_free] and stores
           contiguously.

All matmuls run in fp16 (PSUM accumulates fp32).  Host pre-lays-out
weights/activations so every DMA is contiguous and no on-device
transposes are needed.

The PE stream is the hard floor (4992 matmuls x 512 cycles at the
granted clock) and runs gap-free; measured refinements on top of that:
  - the first matmul starts only once ~2.4MB of input is buffered (big
    first wg slab + x chunks) — starting earlier starves the stream
    and prolongs the cold-clock ramp, costing more than it saves,
  - one PSUM pool serves both phases with phase 2 reusing the phase-1
    tags, so bank recycling is per-slot data dependencies instead of a
    pool-close barrier (kills a ~1.3us transition stall), with the
    last k0's `up` accumulations run i-outer so their SiLU drains
    retire before phase 2 claims the banks,
  - the output is stored fp16 (upcast on host), halving the tail DMA.
"""

import numpy as np

# Problem shapes (hardcoded per spec).
T, H, K = 8192, 4096, 3302
NCORES = 8
KP = 3328                 # K padded to a multiple of 128
NK0 = KP // 128           # 26 k-tiles
NH0 = H // 128            # 32 h-tiles (contraction, phase 1)
TC = T // NCORES          # 1024 tokens per core

_CACHE = {}


def build_nc(kp=KP, h=H, tct=TC, enable_asserts=False):
    """Build + compile the per-core Bass program (SPMD: same on all cores)."""
    from contextlib import ExitStack

    import concourse.mybir as mybir
    import concourse.tile as tile
    from concourse import bacc

    fp16 = mybir.dt.float16
    fp32 = mybir.dt.float32
    Sigmoid = mybir.ActivationFunctionType.Sigmoid

    nk0 = kp // 128
    nh0 = h // 128
    ntf = tct // 512          # moving t-tiles, phase 1
    nt1 = tct // 128          # stationary t-tiles, phase 2
    nhf = h // 512            # moving h-tiles, phase 2

    nc = bacc.Bacc(
        "TRN2", target_bir_lowering=False, debug=False,
        enable_asserts=enable_asserts,
    )
    xt = nc.dram_tensor("xt", [128, nh0 * tct], fp16, kind="ExternalInput").ap()
    wg = nc.dram_tensor("wg", [nk0, 128, nh0 * 128], fp16, kind="ExternalInput").ap()
    wu = nc.dram_tensor("wu", [nk0, 128, nh0 * 128], fp16, kind="ExternalInput").ap()
    wd = nc.dram_tensor("wd", [nk0, 128, h], fp16, kind="ExternalInput").ap()
    out = nc.dram_tensor("out", [tct, h], fp16, kind="ExternalOutput").ap()

    with tile.TileContext(nc) as tc, ExitStack() as ctx:
        xt_pool = ctx.enter_context(tc.tile_pool(name="xtp", bufs=1))
        w_pool = ctx.enter_context(tc.tile_pool(name="wp", bufs=3))
        wd_pool = ctx.enter_context(tc.tile_pool(name="wdp", bufs=8))
        h_pool = ctx.enter_context(tc.tile_pool(name="hp", bufs=1))
        tmp_pool = ctx.enter_context(tc.tile_pool(name="tmpp", bufs=4))
        out_pool = ctx.enter_context(tc.tile_pool(name="outp", bufs=8))
        warm_pool = ctx.enter_context(tc.tile_pool(name="warmp", bufs=1))

        xt_sb = xt_pool.tile([128, nh0 * tct], fp16, name="xt_sb")
        h_sb = h_pool.tile([128, nk0 * tct], fp16, name="h_sb")

        # First k0's gate slab goes ahead of the x^T load so the first
        # matmul's stationary operand lands quickly; the up slab is only
        # needed ~13us later (after the 64 gate matmuls) so it follows
        # the first x^T chunks.
        wg_t0 = w_pool.tile([128, nh0 * 128], fp16, name="wg_t", tag="wg")
        nc.sync.dma_start(wg_t0[:, :], wg[0])
        # x^T in fine chunks: the h0=0 matmul only needs the first chunk.
        nchunk = max(1, (nh0 * tct) // 2048)
        csz = nh0 * tct // nchunk
        for i in range(nchunk):
            nc.sync.dma_start(xt_sb[:, i * csz:(i + 1) * csz],
                              xt[:, i * csz:(i + 1) * csz])
            if i == 1:
                wu_t0 = w_pool.tile([128, nh0 * 128], fp16, name="wu_t", tag="wu")
                nc.sync.dma_start(wu_t0[:, :], wu[0])
        if nchunk <= 1:
            wu_t0 = w_pool.tile([128, nh0 * 128], fp16, name="wu_t", tag="wu")
            nc.sync.dma_start(wu_t0[:, :], wu[0])

        # ---- phase 1: g = x@Wg^T, u = x@Wu^T, h = silu(g)*u ----
        # One PSUM pool serves both phases (phase 2 reuses the phase-1
        # tags): bank recycling is then per-slot data dependencies, not
        # a pool-close barrier, so phase 2's first matmuls don't wait
        # for the last SiLU drain.
        with tc.tile_pool(name="ps1", space="PSUM", bufs=2) as ps1:
            # Warmup matmuls on a zeroed dummy tile fill the otherwise
            # idle PE window while the first weight slab + x chunks are
            # in flight (~7us), so the HAM clock-gate finishes its
            # cold->warm ramp (~3.4us of PE busy) before the first real
            # matmul.  Sized to end before the data arrives; the psum
            # slot (tag pg0) it displaces is drained mid-phase-1, long
            # before phase 2 reclaims it.
            dummy = warm_pool.tile([128, 512], fp16, name="dummy")
            nc.vector.memset(dummy[:, :], 0)
            pwarm = ps1.tile([128, 512], fp32, name="pwarm", tag="pg0")
            for _ in range(12):
                nc.tensor.matmul(pwarm[:, :], dummy[:, 0:128], dummy[:, :],
                                 start=True, stop=True)
            for k0 in range(nk0):
                if k0 == 0:
                    wg_t, wu_t = wg_t0, wu_t0
                else:
                    wg_t = w_pool.tile([128, nh0 * 128], fp16, name="wg_t", tag="wg")
                    nc.sync.dma_start(wg_t[:, :], wg[k0])
                    wu_t = w_pool.tile([128, nh0 * 128], fp16, name="wu_t", tag="wu")
                    nc.sync.dma_start(wu_t[:, :], wu[k0])
                pg = [ps1.tile([128, 512], fp32, name=f"pg{i}", tag=f"pg{i}")
                      for i in range(ntf)]
                pu = [ps1.tile([128, 512], fp32, name=f"pu{i}", tag=f"pu{i}")
                      for i in range(ntf)]
                for h0 in range(nh0):
                    for i in range(ntf):
                        nc.tensor.matmul(
                            pg[i][:, :], wg_t[:, h0 * 128:(h0 + 1) * 128],
                            xt_sb[:, h0 * tct + i * 512:h0 * tct + (i + 1) * 512],
                            start=(h0 == 0), stop=(h0 == nh0 - 1),
                        )
                if k0 == nk0 - 1:
                    # Last iteration: run each pu[i]'s accumulation to
                    # completion before the next (i-outer) so pu[0]'s
                    # SiLU drain — and its PSUM bank — retire well
                    # before phase 2 reuses the banks.
                    for i in range(ntf):
                        for h0 in range(nh0):
                            nc.tensor.matmul(
                                pu[i][:, :], wu_t[:, h0 * 128:(h0 + 1) * 128],
                                xt_sb[:, h0 * tct + i * 512:h0 * tct + (i + 1) * 512],
                                start=(h0 == 0), stop=(h0 == nh0 - 1),
                            )
                else:
                    for h0 in range(nh0):
                        for i in range(ntf):
                            nc.tensor.matmul(
                                pu[i][:, :], wu_t[:, h0 * 128:(h0 + 1) * 128],
                                xt_sb[:, h0 * tct + i * 512:h0 * tct + (i + 1) * 512],
                                start=(h0 == 0), stop=(h0 == nh0 - 1),
                            )
                for i in range(ntf):
                    sg = tmp_pool.tile([128, 512], fp32, name="sg", tag="sg")
                    nc.scalar.activation(sg[:, :], pg[i][:, :], Sigmoid)
                    sl = tmp_pool.tile([128, 512], fp32, name="sl", tag="sl")
                    nc.vector.tensor_mul(sl[:, :], sg[:, :], pg[i][:, :])
                    nc.vector.tensor_mul(
                        h_sb[:, k0 * tct + i * 512:k0 * tct + (i + 1) * 512],
                        sl[:, :], pu[i][:, :])

            # ---- phase 2: out = h @ Wd^T (contract k) ----
            Copy = mybir.ActivationFunctionType.Copy
            ptags = ["pg0", "pg1", "pu0", "pu1"]
            for hf in range(nhf):
                po = [ps1.tile([128, 512], fp32, name=f"po{t1}",
                               tag=ptags[t1 % 4])
                      for t1 in range(nt1)]
                for k0 in range(nk0):
                    wd_t = wd_pool.tile([128, 512], fp16, name="wd_t", tag="wd")
                    nc.sync.dma_start(wd_t[:, :], wd[k0, :, hf * 512:(hf + 1) * 512])
                    for t1 in range(nt1):
                        nc.tensor.matmul(
                            po[t1][:, :],
                            h_sb[:, k0 * tct + t1 * 128:k0 * tct + (t1 + 1) * 128],
                            wd_t[:, :],
                            start=(k0 == 0), stop=(k0 == nk0 - 1),
                        )
                # Drains alternate DVE / ACT so the two engines empty the
                # PSUM banks in parallel and the next hf's matmuls don't
                # stall on bank reuse.  fp16 output halves the store DMA.
                for t1 in range(nt1):
                    ot = out_pool.tile([128, 512], fp16, name="ot", tag="ot")
                    if t1 % 2 == 0:
                        nc.vector.tensor_copy(ot[:, :], po[t1][:, :])
                    else:
                        nc.scalar.activation(ot[:, :], po[t1][:, :], Copy)
                    nc.sync.dma_start(
                        out[t1 * 128:(t1 + 1) * 128, hf * 512:(hf + 1) * 512],
                        ot[:, :])

    nc.compile()
    return nc


def prep_weights(W_gate, W_up, W_down, active_idx, kp=KP, h=H):
    idx = np.asarray(active_idx)
    k = idx.shape[0]
    nk0 = kp // 128
    nh0 = h // 128

    def lay_gu(W):
        a = np.zeros((kp, h), np.float16)
        a[:k] = W[idx].astype(np.float16)
        # [k0, p, h0*128 + k_in] = a[k0*128+k_in, h0*128+p]
        return np.ascontiguousarray(
            a.reshape(nk0, 128, nh0, 128).transpose(0, 3, 2, 1)
        ).reshape(nk0, 128, nh0 * 128)

    wd_a = np.zeros((kp, h), np.float16)
    wd_a[:k] = W_down[:, idx].T.astype(np.float16)
    wd_prep = np.ascontiguousarray(wd_a.reshape(nk0, 128, h))
    return lay_gu(W_gate), lay_gu(W_up), wd_prep


def prep_x_core(xc, h=H, tct=TC):
    nh0 = h // 128
    xt_c = np.ascontiguousarray(
        xc.astype(np.float16).T.reshape(nh0, 128, tct).transpose(1, 0, 2))
    return xt_c.reshape(128, nh0 * tct)


def run(inputs, trace=False, **kw):
    from concourse.bass_utils import run_bass_kernel_spmd

    if "nc" not in _CACHE:
        _CACHE["nc"] = build_nc()
    nc = _CACHE["nc"]

    wg_prep, wu_prep, wd_prep = prep_weights(
        inputs["W_gate"], inputs["W_up"], inputs["W_down"], inputs["active_idx"])
    x = inputs["x"]
    in_maps = [
        {"xt": prep_x_core(x[c * TC:(c + 1) * TC]),
         "wg": wg_prep, "wu": wu_prep, "wd": wd_prep}
        for c in range(NCORES)
    ]
    res = run_bass_kernel_spmd(nc, in_maps, core_ids=list(range(NCORES)),
                               trace=trace, **kw)
    out = np.concatenate(
        [res.results[c]["out"].astype(np.float32) for c in range(NCORES)],
        axis=0)
    return out, res


def kernel(**inputs):
    out, _ = run(inputs, trace=False)
    return out
